# revision 9
# baseline (speedup 1.0000x reference)
"""Trainium2 Bass kernel for block-diagonal (per-frame) multi-head attention.

Reference semantics (fp32):
    q = x@Wq + bq ; k = x@Wk + bk ; v = relu(x@Wv + bv)   (per head, d_head=32)
    scores = (q k^T) / sqrt(32) within each 24-token frame, -inf across frames
    attn = softmax(scores) with +1e-6 in the denominator
    out = attn @ v

v3 design notes:
  - 16 batches data-parallel over 8 cores (2 per core).
  - x enters via a casting SWDGE DMA (fp32->bf16) into a DRAM scratch, then
    xbar-transpose DMAs land xT [d, tok] directly in SBUF: no PE transposes
    and no engine casts at all.  Weights are cast to bf16 in their DMAs.
  - k bias is dropped: (q+bq)@(k+bk)^T = (q+bq)@k^T + (q+bq)@bk^T and the
    second term is constant along the softmax (key) axis, so it cancels in
    the softmax exactly.  q bias rides in as a K=1 ones-row matmul.
  - The softmax eps is dropped: scores/sqrt(32) are O(5) and every frame row
    has 24 valid entries, so the denominator is >= 24*exp(-|s|max) >> 1e-6
    and the eps perturbs the result by < 1e-5 relative.
  - Attention is the baseline dense structure (96x96 group scores, bf16
    mask multiply, K=96 AV with a ones-augmented V producing the softmax
    denominator), which uses only hardware-safe PE tiling patterns
    (same-strip score runs, full-array AV; strip switches always separated
    by full-array matmuls -- back-to-back matmuls on different row strips
    intermittently hang this stack, verified by probes).
"""

import math
from contextlib import ExitStack

import numpy as np

import concourse.bass as bass
from concourse import bacc
import concourse.mybir as mybir
import concourse.tile as tile
from concourse.bass_utils import run_bass_kernel_spmd
from concourse.masks import make_block_diagonal

F32 = mybir.dt.float32
BF16 = mybir.dt.bfloat16
AF = mybir.ActivationFunctionType
ALU = mybir.AluOpType

BS = 16
SEQ = 48
J = 24           # joints (tokens per frame)
N_TOK = SEQ * J  # 1152 tokens per batch
D_IN = 256
H = 8
DH = 32
DM = 256
N_CORES = 8
B2 = BS // N_CORES          # batches per core
TOK = B2 * N_TOK            # 2304 tokens per core
G = 96                      # tokens per attention group (4 frames)
NG = TOK // G               # 24 groups per core
NGB = N_TOK // G            # 12 groups per batch
WAVE = 6                    # groups per score/exp wave
SCALE = 1.0 / math.sqrt(DH)

_CACHE = {}


def _build():
    nc = bacc.Bacc(trn_type="TRN2")

    x_d = nc.dram_tensor("x", [TOK, D_IN], F32, kind="ExternalInput")
    wq_d = nc.dram_tensor("Wq", [D_IN, DM], F32, kind="ExternalInput")
    wk_d = nc.dram_tensor("Wk", [D_IN, DM], F32, kind="ExternalInput")
    wv_d = nc.dram_tensor("Wv", [D_IN, DM], F32, kind="ExternalInput")
    bq_d = nc.dram_tensor("bq", [DM], F32, kind="ExternalInput")
    bk_d = nc.dram_tensor("bk", [DM], F32, kind="ExternalInput")
    bv_d = nc.dram_tensor("bv", [DM], F32, kind="ExternalInput")
    out_d = nc.dram_tensor("out", [TOK, DM], F32, kind="ExternalOutput")

    with tile.TileContext(nc) as tc, ExitStack() as ctx:
        singles = ctx.enter_context(tc.tile_pool(name="singles", bufs=1))
        dram = ctx.enter_context(tc.tile_pool(name="dram", bufs=1, space="DRAM"))
        mmps = ctx.enter_context(tc.tile_pool(name="mmps", bufs=2, space="PSUM"))
        scps = ctx.enter_context(tc.tile_pool(name="scps", bufs=2, space="PSUM"))
        avps = ctx.enter_context(tc.tile_pool(name="avps", bufs=2, space="PSUM"))
        epool = ctx.enter_context(tc.tile_pool(name="epool", bufs=4))
        rpool = ctx.enter_context(tc.tile_pool(name="rpool", bufs=4))

        # ---- x: fp32 -> bf16 cast in the DMA, then xbar transpose to SBUF ----
        xbf_d = dram.tile([TOK, D_IN], BF16)
        xT = singles.tile([128, 2, TOK], BF16, tag="xT")
        XC = 4
        CH = TOK // XC  # 576 tokens per chunk
        nc.gpsimd.dma_start(xbf_d[0:CH, :], x_d[0:CH, :])
        # weights/biases interleave on the gpsimd queue after chunk 0
        w_bf = []
        for wd in (wq_d, wk_d, wv_d):
            wb = singles.tile([128, 2, DM], BF16, tag=f"w_{wd.name}")
            nc.gpsimd.dma_start(wb, wd[:].rearrange("(a p) m -> p a m", p=128))
            w_bf.append(wb)
        wq_bf, wk_bf, wv_bf = w_bf
        bq_row = singles.tile([1, DM], BF16, tag="bq_row")
        nc.gpsimd.dma_start(bq_row, bq_d[None, :])
        bv_row = singles.tile([1, DM], BF16, tag="bv_row")
        nc.gpsimd.dma_start(bv_row, bv_d[None, :])
        for c in range(1, XC):
            nc.gpsimd.dma_start(
                xbf_d[c * CH:(c + 1) * CH, :], x_d[c * CH:(c + 1) * CH, :]
            )
        for c in range(XC):
            for half in range(2):
                nc.sync.dma_start_transpose(
                    xT[:, half, c * CH:(c + 1) * CH],
                    xbf_d[c * CH:(c + 1) * CH, 128 * half:128 * (half + 1)],
                )

        ones512 = singles.tile([1, 512], BF16, tag="ones512")
        nc.vector.memset(ones512, 1.0)
        ones96 = singles.tile([1, G], BF16, tag="ones96")
        nc.vector.memset(ones96, 1.0)

        # block-diagonal 0/1 mask for one 4-frame group, bf16 [96, 96]
        mask = singles.tile([G, G], BF16, tag="mask")
        make_block_diagonal(nc, mask, J)

        # ---- persistent activations ----
        qT = singles.tile([128, 2, TOK], BF16, tag="qT")
        kT = singles.tile([128, 2, TOK], BF16, tag="kT")
        v_aug = singles.tile([G, NG, H, DH + 1], BF16, tag="vaug")
        nc.vector.memset(v_aug[:, :, :, DH:DH + 1], 1.0)
        out_sb = singles.tile([G, NG, DM], F32, tag="out")

        # ---- projections ----
        CHUNKS = [(c, min(512, TOK - c)) for c in range(0, TOK, 512)]
        relu_rot = 0

        def emit_qk_chunk(ci):
            c0, cn = CHUNKS[ci]
            for half in range(2):
                hs = slice(128 * half, 128 * (half + 1))
                psq = mmps.tile([128, 512], F32, tag="mm", name="qproj")
                for kk in range(2):
                    nc.tensor.matmul(
                        psq[:, :cn],
                        lhsT=wq_bf[:, kk, hs],
                        rhs=xT[:, kk, c0:c0 + cn],
                        start=(kk == 0),
                        stop=False,
                    )
                nc.tensor.matmul(
                    psq[:, :cn],
                    lhsT=bq_row[0:1, hs],
                    rhs=ones512[0:1, :cn],
                    start=False,
                    stop=True,
                )
                nc.vector.tensor_copy(qT[:, half, c0:c0 + cn], psq[:, :cn])
                psk = mmps.tile([128, 512], F32, tag="mm", name="kproj")
                for kk in range(2):
                    nc.tensor.matmul(
                        psk[:, :cn],
                        lhsT=wk_bf[:, kk, hs],
                        rhs=xT[:, kk, c0:c0 + cn],
                        start=(kk == 0),
                        stop=(kk == 1),
                    )
                nc.scalar.copy(kT[:, half, c0:c0 + cn], psk[:, :cn])

        def emit_v_group(g):
            nonlocal relu_rot
            vt = mmps.tile([128, 512], F32, tag="mm", name="vproj")
            ps = vt[:G, :DM]
            for kk in range(2):
                nc.tensor.matmul(
                    ps,
                    lhsT=xT[:, kk, g * G:(g + 1) * G],
                    rhs=wv_bf[:, kk, :],
                    start=(kk == 0),
                    stop=False,
                )
            nc.tensor.matmul(
                ps, lhsT=ones96, rhs=bv_row, start=False, stop=True
            )
            src = ps.rearrange("p (h c) -> p h c", h=H)
            dst = v_aug[:, g, :, 0:DH]
            if relu_rot % 2 == 0:
                nc.vector.tensor_scalar_max(dst, src, 0.0)
            else:
                nc.scalar.activation(out=dst, in_=src, func=AF.Relu)
            relu_rot += 1

        for ci in range(3):
            emit_qk_chunk(ci)
        for g in range(NGB):
            emit_v_group(g)
        for ci in range(3, len(CHUNKS)):
            emit_qk_chunk(ci)
        for g in range(NGB, NG):
            emit_v_group(g)

        # ---- attention ----
        out_view = out_d[:].rearrange("(g p) c -> p g c", p=G)
        mask_rot = 0
        for b in range(B2):
            for h in range(H):
                half = h // 4
                hr = (h % 4) * 32
                av = avps.tile([G, NGB, DH + 1], F32, tag="av")
                for w in range(NGB // WAVE):
                    sc = scps.tile([G, WAVE, 128], F32, tag="sc")
                    for gl in range(WAVE):
                        g = w * WAVE + gl
                        c0 = b * N_TOK + g * G
                        nc.tensor.matmul(
                            sc[:, gl, :G],
                            lhsT=kT[hr:hr + 32, half, c0:c0 + G],
                            rhs=qT[hr:hr + 32, half, c0:c0 + G],
                            start=True,
                            stop=True,
                            tile_position=(hr, 0),
                        )
                    eT = epool.tile([G, WAVE, G], BF16, tag="eT")
                    nc.scalar.activation(
                        out=eT, in_=sc[:, :, :G], func=AF.Exp, scale=SCALE
                    )
                    eTm = epool.tile([G, WAVE, G], BF16, tag="eTm")
                    mask_eng = nc.vector if mask_rot % 8 < 5 else nc.gpsimd
                    mask_rot += 1
                    mask_eng.tensor_tensor(
                        eTm,
                        eT,
                        mask[:, None, :].to_broadcast((G, WAVE, G)),
                        ALU.mult,
                    )
                    for gl in range(WAVE):
                        g = w * WAVE + gl
                        nc.tensor.matmul(
                            av[:, g, :],
                            lhsT=eTm[:, gl, :],
                            rhs=v_aug[:, b * NGB + g, h, :],
                            start=True,
                            stop=True,
                        )
                # normalize: out = av[:, :, :32] / av[:, :, 32]   (eps dropped)
                rt = rpool.tile([G, NGB], F32, tag="rt")
                nc.vector.reciprocal(rt, av[:, :, DH])
                nc.vector.tensor_tensor(
                    out_sb[:, b * NGB:(b + 1) * NGB, h * DH:(h + 1) * DH],
                    av[:, :, 0:DH],
                    rt[:, :, None].to_broadcast((G, NGB, DH)),
                    ALU.mult,
                )
                dma_eng = nc.sync if h % 2 == 0 else nc.scalar
                dma_eng.dma_start(
                    out_view[:, b * NGB:(b + 1) * NGB, h * DH:(h + 1) * DH],
                    out_sb[:, b * NGB:(b + 1) * NGB, h * DH:(h + 1) * DH],
                )

    nc.compile()
    return nc


def _get_nc():
    if "nc" not in _CACHE:
        _CACHE["nc"] = _build()
    return _CACHE["nc"]


def _run(inputs, **kw):
    nc = _get_nc()
    x = np.ascontiguousarray(inputs["x"], dtype=np.float32)
    shared = {
        k: np.ascontiguousarray(inputs[k], dtype=np.float32)
        for k in ("Wq", "Wk", "Wv", "bq", "bk", "bv")
    }
    in_maps = []
    for c in range(N_CORES):
        m = dict(shared)
        m["x"] = np.ascontiguousarray(
            x[c * B2:(c + 1) * B2].reshape(TOK, D_IN)
        )
        in_maps.append(m)
    res = run_bass_kernel_spmd(nc, in_maps, core_ids=list(range(N_CORES)), **kw)
    out = np.concatenate(
        [r["out"].reshape(B2, N_TOK, DM) for r in res.results], axis=0
    )
    return out, res


def kernel(**inputs) -> np.ndarray:
    out, _ = _run(inputs)
    return out


# revision 12
# speedup vs baseline: 1.0083x; 1.0083x over previous
"""Trainium2 Bass kernel for block-diagonal (per-frame) multi-head attention.

Reference semantics (fp32):
    q = x@Wq + bq ; k = x@Wk + bk ; v = relu(x@Wv + bv)   (per head, d_head=32)
    scores = (q k^T) / sqrt(32) within each 24-token frame, -inf across frames
    attn = softmax(scores) with +1e-6 in the denominator
    out = attn @ v

v3 design notes:
  - 16 batches data-parallel over 8 cores (2 per core).
  - x enters via a casting SWDGE DMA (fp32->bf16) into a DRAM scratch, then
    xbar-transpose DMAs land xT [d, tok] directly in SBUF: no PE transposes
    and no engine casts at all.  Weights are cast to bf16 in their DMAs.
  - k bias is dropped: (q+bq)@(k+bk)^T = (q+bq)@k^T + (q+bq)@bk^T and the
    second term is constant along the softmax (key) axis, so it cancels in
    the softmax exactly.  q bias rides in as a K=1 ones-row matmul.
  - The softmax eps is dropped: scores/sqrt(32) are O(5) and every frame row
    has 24 valid entries, so the denominator is >= 24*exp(-|s|max) >> 1e-6
    and the eps perturbs the result by < 1e-5 relative.
  - Attention is the baseline dense structure (96x96 group scores, bf16
    mask multiply, K=96 AV with a ones-augmented V producing the softmax
    denominator), which uses only hardware-safe PE tiling patterns
    (same-strip score runs, full-array AV; strip switches always separated
    by full-array matmuls -- back-to-back matmuls on different row strips
    intermittently hang this stack, verified by probes).
"""

import math
from contextlib import ExitStack

import numpy as np

import concourse.bass as bass
from concourse import bacc
import concourse.mybir as mybir
import concourse.tile as tile
from concourse.bass_utils import run_bass_kernel_spmd
from concourse.masks import make_block_diagonal

F32 = mybir.dt.float32
BF16 = mybir.dt.bfloat16
AF = mybir.ActivationFunctionType
ALU = mybir.AluOpType

BS = 16
SEQ = 48
J = 24           # joints (tokens per frame)
N_TOK = SEQ * J  # 1152 tokens per batch
D_IN = 256
H = 8
DH = 32
DM = 256
N_CORES = 8
B2 = BS // N_CORES          # batches per core
TOK = B2 * N_TOK            # 2304 tokens per core
G = 96                      # tokens per attention group (4 frames)
NG = TOK // G               # 24 groups per core
NGB = N_TOK // G            # 12 groups per batch
WAVE = 6                    # groups per score/exp wave
SCALE = 1.0 / math.sqrt(DH)

_CACHE = {}


def _build():
    nc = bacc.Bacc(trn_type="TRN2")

    x_d = nc.dram_tensor("x", [TOK, D_IN], F32, kind="ExternalInput")
    wq_d = nc.dram_tensor("Wq", [D_IN, DM], F32, kind="ExternalInput")
    wk_d = nc.dram_tensor("Wk", [D_IN, DM], F32, kind="ExternalInput")
    wv_d = nc.dram_tensor("Wv", [D_IN, DM], F32, kind="ExternalInput")
    bq_d = nc.dram_tensor("bq", [DM], F32, kind="ExternalInput")
    bk_d = nc.dram_tensor("bk", [DM], F32, kind="ExternalInput")
    bv_d = nc.dram_tensor("bv", [DM], F32, kind="ExternalInput")
    out_d = nc.dram_tensor("out", [TOK, DM], F32, kind="ExternalOutput")

    with tile.TileContext(nc) as tc, ExitStack() as ctx:
        singles = ctx.enter_context(tc.tile_pool(name="singles", bufs=1))
        dram = ctx.enter_context(tc.tile_pool(name="dram", bufs=1, space="DRAM"))
        mmps = ctx.enter_context(tc.tile_pool(name="mmps", bufs=2, space="PSUM"))
        scps = ctx.enter_context(tc.tile_pool(name="scps", bufs=2, space="PSUM"))
        avps = ctx.enter_context(tc.tile_pool(name="avps", bufs=2, space="PSUM"))
        epool = ctx.enter_context(tc.tile_pool(name="epool", bufs=4))
        rpool = ctx.enter_context(tc.tile_pool(name="rpool", bufs=4))

        # ---- x: fp32 -> bf16 cast in the DMA, then xbar transpose to SBUF ----
        xT = singles.tile([128, 2, TOK], BF16, tag="xT")
        XC = 4
        CH = TOK // XC  # 576 tokens per chunk
        xbf_c = []
        for c in range(XC):
            xb = dram.tile([CH, D_IN], BF16, tag=f"xbf{c}", name=f"xbf{c}")
            xbf_c.append(xb)
        nc.gpsimd.dma_start(xbf_c[0][:], x_d[0:CH, :])
        # weights/biases interleave on the gpsimd queue after chunk 0
        w_bf = []
        for wd in (wq_d, wk_d, wv_d):
            wb = singles.tile([128, 2, DM], BF16, tag=f"w_{wd.name}")
            nc.gpsimd.dma_start(wb, wd[:].rearrange("(a p) m -> p a m", p=128))
            w_bf.append(wb)
        wq_bf, wk_bf, wv_bf = w_bf
        bq_row = singles.tile([1, DM], BF16, tag="bq_row")
        nc.gpsimd.dma_start(bq_row, bq_d[None, :])
        bv_row = singles.tile([1, DM], BF16, tag="bv_row")
        nc.gpsimd.dma_start(bv_row, bv_d[None, :])
        for c in range(1, XC):
            nc.gpsimd.dma_start(xbf_c[c][:], x_d[c * CH:(c + 1) * CH, :])
        for c in range(XC):
            for half in range(2):
                eng = nc.sync if half == 0 else nc.scalar
                eng.dma_start_transpose(
                    xT[:, half, c * CH:(c + 1) * CH],
                    xbf_c[c][:, 128 * half:128 * (half + 1)],
                )

        ones512 = singles.tile([1, 512], BF16, tag="ones512")
        nc.vector.memset(ones512, 1.0)
        ones96 = singles.tile([1, G], BF16, tag="ones96")
        nc.vector.memset(ones96, 1.0)

        # block-diagonal 0/1 mask for one 4-frame group, bf16 [96, 96]
        mask = singles.tile([G, G], BF16, tag="mask")
        make_block_diagonal(nc, mask, J)

        # ---- persistent activations ----
        qT = singles.tile([128, 2, TOK], BF16, tag="qT")
        kT = singles.tile([128, 2, TOK], BF16, tag="kT")
        v_aug = singles.tile([G, NG, H, DH + 1], BF16, tag="vaug")
        nc.vector.memset(v_aug[:, :, :, DH:DH + 1], 1.0)
        out_sb = singles.tile([G, NG, DM], F32, tag="out")

        # ---- projections ----
        CHUNKS = [(c, min(512, TOK - c)) for c in range(0, TOK, 512)]
        relu_rot = 0

        def emit_qk_chunk(ci):
            c0, cn = CHUNKS[ci]
            for half in range(2):
                hs = slice(128 * half, 128 * (half + 1))
                psq = mmps.tile([128, 512], F32, tag="mm", name="qproj")
                for kk in range(2):
                    nc.tensor.matmul(
                        psq[:, :cn],
                        lhsT=wq_bf[:, kk, hs],
                        rhs=xT[:, kk, c0:c0 + cn],
                        start=(kk == 0),
                        stop=False,
                    )
                nc.tensor.matmul(
                    psq[:, :cn],
                    lhsT=bq_row[0:1, hs],
                    rhs=ones512[0:1, :cn],
                    start=False,
                    stop=True,
                )
                nc.vector.tensor_copy(qT[:, half, c0:c0 + cn], psq[:, :cn])
                psk = mmps.tile([128, 512], F32, tag="mm", name="kproj")
                for kk in range(2):
                    nc.tensor.matmul(
                        psk[:, :cn],
                        lhsT=wk_bf[:, kk, hs],
                        rhs=xT[:, kk, c0:c0 + cn],
                        start=(kk == 0),
                        stop=(kk == 1),
                    )
                nc.scalar.copy(kT[:, half, c0:c0 + cn], psk[:, :cn])

        def emit_v_group(g):
            nonlocal relu_rot
            vt = mmps.tile([128, 512], F32, tag="mm", name="vproj")
            ps = vt[:G, :DM]
            for kk in range(2):
                nc.tensor.matmul(
                    ps,
                    lhsT=xT[:, kk, g * G:(g + 1) * G],
                    rhs=wv_bf[:, kk, :],
                    start=(kk == 0),
                    stop=False,
                )
            nc.tensor.matmul(
                ps, lhsT=ones96, rhs=bv_row, start=False, stop=True
            )
            src = ps.rearrange("p (h c) -> p h c", h=H)
            dst = v_aug[:, g, :, 0:DH]
            if relu_rot % 2 == 0:
                nc.vector.tensor_scalar_max(dst, src, 0.0)
            else:
                nc.scalar.activation(out=dst, in_=src, func=AF.Relu)
            relu_rot += 1

        for ci in range(3):
            emit_qk_chunk(ci)
        for g in range(NGB):
            emit_v_group(g)
        for ci in range(3, len(CHUNKS)):
            emit_qk_chunk(ci)
        for g in range(NGB, NG):
            emit_v_group(g)

        # ---- attention ----
        out_view = out_d[:].rearrange("(g p) c -> p g c", p=G)
        mask_rot = 0
        for b in range(B2):
            for h in range(H):
                half = h // 4
                hr = (h % 4) * 32
                av = avps.tile([G, NGB, DH + 8], F32, tag="av")
                for w in range(NGB // WAVE):
                    sc = scps.tile([G, WAVE, 128], F32, tag="sc")
                    for gl in range(WAVE):
                        g = w * WAVE + gl
                        c0 = b * N_TOK + g * G
                        nc.tensor.matmul(
                            sc[:, gl, :G],
                            lhsT=kT[hr:hr + 32, half, c0:c0 + G],
                            rhs=qT[hr:hr + 32, half, c0:c0 + G],
                            start=True,
                            stop=True,
                            tile_position=(hr, 0),
                        )
                    eT = epool.tile([G, WAVE, G], BF16, tag="eT")
                    nc.scalar.activation(
                        out=eT, in_=sc[:, :, :G], func=AF.Exp, scale=SCALE
                    )
                    eTm = epool.tile([G, WAVE, G], BF16, tag="eTm")
                    mask_eng = nc.vector if mask_rot % 8 < 5 else nc.gpsimd
                    mask_rot += 1
                    mask_eng.tensor_tensor(
                        eTm,
                        eT,
                        mask[:, None, :].to_broadcast((G, WAVE, G)),
                        ALU.mult,
                    )
                    for gl in range(WAVE):
                        g = w * WAVE + gl
                        nc.tensor.matmul(
                            av[:, g, :DH + 1],
                            lhsT=eTm[:, gl, :],
                            rhs=v_aug[:, b * NGB + g, h, :],
                            start=True,
                            stop=True,
                        )
                # normalize: out = av[:, :, :32] / av[:, :, 32]   (eps dropped)
                rt = rpool.tile([G, NGB], F32, tag="rt")
                nc.vector.reciprocal(rt, av[:, :, DH])
                nc.vector.tensor_tensor(
                    out_sb[:, b * NGB:(b + 1) * NGB, h * DH:(h + 1) * DH],
                    av[:, :, 0:DH],
                    rt[:, :, None].to_broadcast((G, NGB, DH)),
                    ALU.mult,
                )
                dma_eng = nc.sync if h % 2 == 0 else nc.scalar
                dma_eng.dma_start(
                    out_view[:, b * NGB:(b + 1) * NGB, h * DH:(h + 1) * DH],
                    out_sb[:, b * NGB:(b + 1) * NGB, h * DH:(h + 1) * DH],
                )

    nc.compile()
    return nc


def _get_nc():
    if "nc" not in _CACHE:
        _CACHE["nc"] = _build()
    return _CACHE["nc"]


def _run(inputs, **kw):
    nc = _get_nc()
    x = np.ascontiguousarray(inputs["x"], dtype=np.float32)
    shared = {
        k: np.ascontiguousarray(inputs[k], dtype=np.float32)
        for k in ("Wq", "Wk", "Wv", "bq", "bk", "bv")
    }
    in_maps = []
    for c in range(N_CORES):
        m = dict(shared)
        m["x"] = np.ascontiguousarray(
            x[c * B2:(c + 1) * B2].reshape(TOK, D_IN)
        )
        in_maps.append(m)
    res = run_bass_kernel_spmd(nc, in_maps, core_ids=list(range(N_CORES)), **kw)
    out = np.concatenate(
        [r["out"].reshape(B2, N_TOK, DM) for r in res.results], axis=0
    )
    return out, res


def kernel(**inputs) -> np.ndarray:
    out, _ = _run(inputs)
    return out
